# revision 13
# baseline (speedup 1.0000x reference)
"""Trainium2 Bass kernel for a ragged-sequence RNN classifier.

Model (see original nn.Module): tokens are consumed right-aligned in reverse
order; at step t samples with length >= T-t are active. h starts at 0 and is
updated as h = tanh(emb @ W_ih.T + b_ih + h @ W_hh.T + b_hh) for active rows.
Then MLP head: log_softmax(relu(relu(h@l0+b0)@l1+b1)).

Key restructuring:
  * Activity over time is a step function (once active, always active), and
    h starts at 0.  If the per-step input projection P_t is exactly 0 for
    inactive (t,b), then h = tanh(h@W_hh.T + P_t) densely reproduces the
    masked scan (tanh(0)=0 keeps h at 0 until the sample activates).
  * P_t = emb@W_ih.T + (b_ih+b_hh) is made 0 for inactive tokens by routing
    their embedding gather to an all-zero table row, with the bias folded in
    as an extra "constant 1" embedding column (so the zero row also zeroes
    the bias).
  * Data-parallel over batch: 8 cores x 64 rows; the T=128 scan is local.
  * Per core, at most 64*128=8192 distinct tokens are referenced, so the
    host builds a compacted bf16 embedding table via np.unique; the remapped
    indices fit the int16 index requirement of dma_gather, which also
    transposes rows into the [d, token] layout needed as matmul rhs.
  * Everything on-chip is in a transposed [feature, batch] layout so the
    sequential scan needs no transposes: out[j,b] += W_hh.T[k,j]^T h[k,b].

All matmul operands are bf16 (fp32 PSUM accumulation).  The network output
is log_softmax over 3 near-uniform tiny logits (weights are ~N(0, 0.02^2)),
so bf16 rounding perturbs the output by ~1e-5 absolute - far inside any
reasonable tolerance.
"""

import os
import numpy as np
import ml_dtypes

import concourse.bass as bass
import concourse.bacc as bacc
from concourse import mybir, tile
from concourse import bass_utils
from concourse.alu_op_type import AluOpType

BF16 = mybir.dt.float16  # 16-bit matmul dtype (fp16: 11-bit mantissa)
F32 = mybir.dt.float32
I16 = mybir.dt.int16
AF = mybir.ActivationFunctionType
NPBF16 = np.float16

# Problem sizes (hardcoded per the harness contract).
B, T = 512, 128
V, D, H, MLP, C = 50000, 300, 512, 1024, 3
NCORES = 8
BL = B // NCORES            # 64 local batch rows
DP = 384                    # padded embedding dim (3 x 128); col 300 = bias 1s
NTOK = T * BL               # 8192 tokens per core in scan order n = t*BL + b
NT = 512                    # tokens per phase-1 tile
NTILES = NTOK // NT         # 16
TBL = 8320                  # compacted table rows (<= 8192 used + zero rows)
ZROW = TBL - 1              # guaranteed all-zero row for inactive tokens
KC = H // 128               # 4 hidden chunks
DC = DP // 128              # 3 embedding chunks
MC = MLP // 128             # 8 mlp chunks
STEPS_PER_PTILE = NT // BL  # 8


def _build_program(debug_dumps=False):
    nc = bacc.Bacc("TRN2", target_bir_lowering=False, debug=False)
    dbg = {}
    if debug_dumps:
        dbg["P0"] = nc.dram_tensor("dbg_P0", [128, KC, NT], F32, kind="ExternalOutput")
        dbg["P15"] = nc.dram_tensor("dbg_P15", [128, KC, NT], F32, kind="ExternalOutput")
        dbg["emb0"] = nc.dram_tensor("dbg_emb0", [128, DC, NT], BF16, kind="ExternalOutput")
        dbg["h"] = nc.dram_tensor("dbg_h", [128, KC, BL], BF16, kind="ExternalOutput")
        dbg["h8"] = nc.dram_tensor("dbg_h8", [128, KC, BL], BF16, kind="ExternalOutput")
        dbg["aT"] = nc.dram_tensor("dbg_aT", [128, MC, BL], BF16, kind="ExternalOutput")

    etab_d = nc.dram_tensor("etab", [TBL, DP], BF16, kind="ExternalInput")
    idx_d = nc.dram_tensor("idx", [128, NTOK // 16], I16, kind="ExternalInput")
    wih_d = nc.dram_tensor("wih", [128, DC, H], BF16, kind="ExternalInput")
    whh_d = nc.dram_tensor("whh", [128, KC, H], BF16, kind="ExternalInput")
    l0w_d = nc.dram_tensor("l0w", [128, KC, MLP], BF16, kind="ExternalInput")
    l1w_d = nc.dram_tensor("l1w", [128, MC, C], BF16, kind="ExternalInput")
    l0b_d = nc.dram_tensor("l0b", [128, MC], F32, kind="ExternalInput")
    l1b_d = nc.dram_tensor("l1b", [BL, C], F32, kind="ExternalInput")
    out_d = nc.dram_tensor("out", [BL, C], F32, kind="ExternalOutput")

    with tile.TileContext(nc) as tc:
        with (
            tc.tile_pool(name="const", bufs=1) as cp,
            tc.tile_pool(name="hbuf", bufs=2) as hp,
            tc.tile_pool(name="embt", bufs=4) as ep,
            tc.tile_pool(name="tmp", bufs=4) as tp,
            tc.tile_pool(name="ps1", bufs=8, space="PSUM") as pp1,
        ):
            # --- resident weights/indices ---
            wih = cp.tile([128, DC, H], BF16)
            whh = cp.tile([128, KC, H], BF16)
            l0w = cp.tile([128, KC, MLP], BF16)
            l1w = cp.tile([128, MC, C], BF16)
            l0b = cp.tile([128, MC], F32)
            l1b = cp.tile([BL, C], F32)
            idx = cp.tile([128, NTOK // 16], I16)
            nc.sync.dma_start(wih[:], wih_d.ap())
            nc.sync.dma_start(whh[:], whh_d.ap())
            nc.sync.dma_start(l0w[:], l0w_d.ap())
            nc.sync.dma_start(l1w[:], l1w_d.ap())
            nc.sync.dma_start(l0b[:], l0b_d.ap())
            nc.sync.dma_start(l1b[:], l1b_d.ap())
            nc.sync.dma_start(idx[:], idx_d.ap())

            # input projections P, one tile per phase-1 n-tile
            ptiles = [
                cp.tile([128, KC, NT], F32, tag=f"P{i}", name=f"P{i}")
                for i in range(NTILES)
            ]

            # --- phase 1: gather + project  P[j, n] = sum_d WihT[d,j] embT[d,n]
            for nt in range(NTILES):
                embT = ep.tile([128, DC, NT], BF16, tag="embT")
                nc.gpsimd.dma_gather(
                    out_ap=embT[:, :, :],
                    in_ap=etab_d.ap(),
                    idxs_ap=idx[:, nt * (NT // 16):(nt + 1) * (NT // 16)],
                    num_idxs=NT,
                    num_idxs_reg=NT,
                    elem_size=DP,
                    transpose=True,
                )
                for jc in range(KC):
                    ps = pp1.tile([128, NT], F32, tag="ps")
                    for dc in range(DC):
                        nc.tensor.matmul(
                            ps[:],
                            wih[:, dc, jc * 128:(jc + 1) * 128],
                            embT[:, dc, :],
                            start=(dc == 0),
                            stop=(dc == DC - 1),
                        )
                    nc.scalar.copy(ptiles[nt][:, jc, :], ps[:])
                if debug_dumps and nt == 0:
                    nc.sync.dma_start(dbg["emb0"].ap(), embT[:])

            # --- phase 2: the scan  h = tanh(h @ W_hh.T + P_t) ---
            h = hp.tile([128, KC, BL], BF16, tag="h")
            nc.gpsimd.memset(h[:], 0.0)
            for t in range(T):
                if debug_dumps and t == 8:
                    nc.sync.dma_start(dbg["h8"].ap(), h[:])
                pt = ptiles[t // STEPS_PER_PTILE]
                col = (t % STEPS_PER_PTILE) * BL
                pss = [
                    pp1.tile([128, BL], F32, tag="ps", name=f"sps{t}_{j}")
                    for j in range(KC)
                ]
                for kc in range(KC):
                    for jc in range(KC):
                        nc.tensor.matmul(
                            pss[jc][:],
                            whh[:, kc, jc * 128:(jc + 1) * 128],
                            h[:, kc, :],
                            start=(kc == 0),
                            stop=(kc == KC - 1),
                        )
                hn = hp.tile([128, KC, BL], BF16, tag="h")
                for jc in range(KC):
                    tmp = tp.tile([128, BL], F32, tag="tmp")
                    nc.vector.tensor_add(
                        tmp[:], pss[jc][:], pt[:, jc, col:col + BL]
                    )
                    nc.scalar.activation(hn[:, jc, :], tmp[:], AF.Tanh)
                h = hn

            if debug_dumps:
                nc.sync.dma_start(dbg["P0"].ap(), ptiles[0][:])
                nc.sync.dma_start(dbg["P15"].ap(), ptiles[15][:])
                nc.sync.dma_start(dbg["h"].ap(), h[:])

            # --- phase 3: MLP head + log_softmax ---
            aT = cp.tile([128, MC, BL], BF16)
            for mc in range(MC):
                ps = pp1.tile([128, BL], F32, tag="ps")
                for jc in range(KC):
                    nc.tensor.matmul(
                        ps[:],
                        l0w[:, jc, mc * 128:(mc + 1) * 128],
                        h[:, jc, :],
                        start=(jc == 0),
                        stop=(jc == KC - 1),
                    )
                nc.scalar.activation(
                    aT[:, mc, :], ps[:], AF.Relu, bias=l0b[:, mc:mc + 1]
                )
            if debug_dumps:
                nc.sync.dma_start(dbg["aT"].ap(), aT[:])
            psl = pp1.tile([BL, C], F32, tag="ps")
            for mc in range(MC):
                nc.tensor.matmul(
                    psl[:],
                    aT[:, mc, :],
                    l1w[:, mc, :],
                    start=(mc == 0),
                    stop=(mc == MC - 1),
                )
            lg = tp.tile([BL, C], F32, tag="lg")
            nc.vector.tensor_add(lg[:], psl[:], l1b[:])
            nc.vector.tensor_scalar_max(lg[:], lg[:], 0.0)
            mx = tp.tile([BL, 1], F32, tag="mx")
            nc.vector.tensor_reduce(
                mx[:], lg[:], axis=mybir.AxisListType.X, op=AluOpType.max
            )
            sh = tp.tile([BL, C], F32, tag="sh")
            nc.vector.tensor_scalar_sub(sh[:], lg[:], mx[:])
            ex = tp.tile([BL, C], F32, tag="ex")
            nc.scalar.activation(ex[:], sh[:], AF.Exp)
            sm = tp.tile([BL, 1], F32, tag="sm")
            nc.vector.tensor_reduce(
                sm[:], ex[:], axis=mybir.AxisListType.X, op=AluOpType.add
            )
            ls = tp.tile([BL, 1], F32, tag="ls")
            nc.scalar.activation(ls[:], sm[:], AF.Ln)
            ou = tp.tile([BL, C], F32, tag="ou")
            nc.vector.tensor_scalar_sub(ou[:], sh[:], ls[:])
            nc.sync.dma_start(out_d.ap(), ou[:])

    nc.compile()
    return nc


def make_in_maps(x, lengths, E, W_ih, b_ih, W_hh, b_hh, l0_w, l0_b, l1_w, l1_b):
    x = np.asarray(x)
    lengths = np.asarray(lengths)
    E = np.asarray(E, np.float32)
    bhb = (np.asarray(b_ih, np.float32) + np.asarray(b_hh, np.float32))

    wihT = np.zeros((DP, H), np.float32)
    wihT[:D] = np.asarray(W_ih, np.float32).T
    wihT[D] = bhb  # bias folded against the constant-1 embedding column
    wih_in = np.ascontiguousarray(
        wihT.reshape(DC, 128, H).transpose(1, 0, 2)
    ).astype(NPBF16)
    whh_in = np.ascontiguousarray(
        np.asarray(W_hh, np.float32).T.reshape(KC, 128, H).transpose(1, 0, 2)
    ).astype(NPBF16)
    l0w_in = np.ascontiguousarray(
        np.asarray(l0_w, np.float32).T.reshape(KC, 128, MLP).transpose(1, 0, 2)
    ).astype(NPBF16)
    l1w_in = np.ascontiguousarray(
        np.asarray(l1_w, np.float32).T.reshape(MC, 128, C).transpose(1, 0, 2)
    ).astype(NPBF16)
    l0b_in = np.ascontiguousarray(
        np.asarray(l0_b, np.float32).reshape(MC, 128).T
    )
    l1b_in = np.ascontiguousarray(
        np.broadcast_to(np.asarray(l1_b, np.float32), (BL, C))
    )

    rev = np.arange(T)[::-1]
    in_maps = []
    for c in range(NCORES):
        xs = x[c * BL:(c + 1) * BL]          # [BL, T]
        lsl = lengths[c * BL:(c + 1) * BL]   # [BL]
        toks = xs[:, ::-1].T                 # [T, BL]; token consumed at step t
        act = rev[:, None] < lsl[None, :]    # [T, BL]
        uniq, inv = np.unique(toks, return_inverse=True)
        inv = inv.reshape(toks.shape)
        tab = np.zeros((TBL, DP), NPBF16)
        tab[:len(uniq), :D] = E[uniq].astype(NPBF16)
        tab[:len(uniq), D] = np.float16(1.0)
        idxs = np.where(act, inv, ZROW).astype(np.int16).reshape(-1)
        # wrapped [16, NTOK/16] and replicated across all 8 16-partition
        # groups: the Q7 tx/rx cpu pair of each SWDGE queue reads indices
        # from its own partition window.
        idx_in = np.ascontiguousarray(
            np.tile(idxs.reshape(NTOK // 16, 16).T, (8, 1))
        )
        in_maps.append({
            "etab": tab,
            "idx": idx_in,
            "wih": wih_in,
            "whh": whh_in,
            "l0w": l0w_in,
            "l1w": l1w_in,
            "l0b": l0b_in,
            "l1b": l1b_in,
        })
    return in_maps


_NC_CACHE = []


def _get_nc():
    if not _NC_CACHE:
        _NC_CACHE.append(_build_program())
    return _NC_CACHE[0]


def kernel(x, lengths, E, W_ih, b_ih, W_hh, b_hh, l0_w, l0_b, l1_w, l1_b):
    assert np.asarray(x).shape == (B, T)
    in_maps = make_in_maps(
        x, lengths, E, W_ih, b_ih, W_hh, b_hh, l0_w, l0_b, l1_w, l1_b
    )
    nc = _get_nc()
    trace = bool(int(os.environ.get("KERNEL_TRACE", "0")))
    from concourse.bass_interp import get_hw_module

    old_m = nc.m
    nc.m = get_hw_module(nc.m)
    try:
        res = bass_utils.run_bass_kernel_spmd(
            nc, in_maps, core_ids=list(range(NCORES)), trace=trace
        )
    finally:
        nc.m = old_m
    if trace:
        kernel.last_result = res
    out = np.concatenate(
        [res.results[c]["out"] for c in range(NCORES)], axis=0
    ).astype(np.float32)
    return out
